# revision 17
# baseline (speedup 1.0000x reference)
"""Trainium2 Bass kernel for DualTierMiras (dual low-rank tier read + LayerNorm-gate mix).

Computes, for k [N, d]:
    v_t   = k @ (SCALE * tanh(B_t @ C_t.T) + diag(D_t)).T      (t in {fast, deep})
    h     = LayerNorm(k) * gamma + beta
    w     = sigmoid(silu(h @ W1.T + b1) @ W2.T + b2 + base_logit)
    out   = w * v_fast + (1 - w) * v_deep

Strategy: data-parallel over rows across 8 NeuronCores; all device matmuls
contract over d, so tensors are kept transposed ([d, rows]).

Fast path (lowrank linearization valid, D == 0):
  * tanh(u) ~= u (max|B C^T| tiny, checked on host) -> tier reads are a
    single fused K=64 matmul per out tile with the gate folded in (Gw trick).
  * The gate GEMM h @ W1.T runs in fp8(e4m3) with DoubleRow perf mode
    (K=256 per pass).  LN's affine (gamma, beta) is folded into W1/b1 on the
    host; the per-token -mu*rstd correction becomes one K=1 matmul row with
    the host-precomputed column-sum vector c_o = sum_j W1_oj, so the device
    only computes z' = k * rstd (one DVE op per tile) before the matmul.
  * LN stats: sums ride along in the G matmul (ones column in C_aug);
    sum-of-squares accumulates via fp8 DoubleRow with a ones stationary.
  * sigmoid(x) == 0.5*tanh(x/2)+0.5 keeps the whole kernel inside a single
    activation-function table set (silu/tanh/copy); sqrt's set is preloaded
    with a dummy op during the DMA prologue.
  * W1 is host-retiled so every DMA line is >= 512B contiguous (full BW).

Fallback path ("tanh" mode or D != 0): the previous-generation bf16 kernel,
kept verbatim below.
"""

from contextlib import ExitStack

import numpy as np

N, D, R = 8192, 2048, 32
NCORES = 8
NSH = N // NCORES          # rows per core
P = 128                    # SBUF partitions
NJ = D // P                # 16 chunks of d
FH = 512                   # free-dim half of NSH (PSUM bank width in fp32)
NH = NSH // FH             # 2 halves
NA = NJ // 2               # 8 K=256 pairs for DoubleRow
SCALE = 0.1
LN_EPS = 1e-5
W1_SCALE = 32.0            # fp8 pre-scale for W1 (power of two)
LOWRANK_THR = 0.10

_NC_CACHE: dict = {}


# ------------------------------------------------------------- fast device build

def build_nc_fast(repeat: int = 1, sim_safe: bool = False):
    import concourse.bacc as bacc
    import concourse.tile as tile
    from concourse import mybir

    f32 = mybir.dt.float32
    bf16 = mybir.dt.bfloat16
    fp8 = mybir.dt.float8e4
    nc = bacc.Bacc("TRN2", target_bir_lowering=False, debug=False,
                   num_devices=NCORES)

    kt_d = nc.dram_tensor("kt", [D, NSH], bf16, kind="ExternalInput")
    w18_d = nc.dram_tensor("w18", [D, D], fp8, kind="ExternalInput")
    w1x_d = nc.dram_tensor("w1x", [P, 2 * D], fp8, kind="ExternalInput")
    caug_d = nc.dram_tensor("caug", [P, NJ * 65], bf16, kind="ExternalInput")
    pv_d = nc.dram_tensor("pv", [P, 32], f32, kind="ExternalInput")
    sc2_d = nc.dram_tensor("sc2", [1, 1], f32, kind="ExternalInput")
    bt_d = nc.dram_tensor("bt", [64, D], bf16, kind="ExternalInput")
    out_d = nc.dram_tensor("outT", [D, NSH], bf16, kind="ExternalOutput")

    with tile.TileContext(nc) as tc:
        for _ in range(repeat):
            with ExitStack() as ctx:
                _emit_fast(ctx, tc, nc, kt_d, w18_d, w1x_d, caug_d, pv_d,
                           sc2_d, bt_d, out_d, sim_safe=sim_safe)
    nc.compile()
    return nc


def _emit_fast(ctx, tc, nc, kt_d, w18_d, w1x_d, caug_d, pv_d, sc2_d, bt_d,
               out_d, sim_safe=False):
    import os

    from concourse import mybir

    f32 = mybir.dt.float32
    bf16 = mybir.dt.bfloat16
    fp8 = mybir.dt.float8e4
    AF = mybir.ActivationFunctionType
    ALU = mybir.AluOpType
    DR = mybir.MatmulPerfMode.DoubleRow
    # gpsimd (Pool engine) takes the squares and half the z-prep; flag to
    # fall back to DVE if it misbehaves on hardware.
    gp = nc.vector if os.environ.get("K_NO_GPSIMD") else nc.gpsimd

    const = ctx.enter_context(tc.tile_pool(name="const", bufs=1))
    persist = ctx.enter_context(tc.tile_pool(name="persist", bufs=1))
    sq8p = ctx.enter_context(tc.tile_pool(name="sq8p", bufs=2))
    h2pool = ctx.enter_context(tc.tile_pool(name="h2p", bufs=3))
    outpool = ctx.enter_context(tc.tile_pool(name="outp", bufs=4))
    svec = ctx.enter_context(tc.tile_pool(name="svec", bufs=4))
    # bcast psum: one [P, FH] rotating bank, alive across phases
    psBC = ctx.enter_context(tc.tile_pool(name="psBC", bufs=1, space="PSUM"))

    # ---- tiny consts + act-table warm + PE warm ---------------------------
    pv = const.tile([P, 32], f32, tag="pv", name="pv")
    nc.sync.dma_start(pv[:], pv_d[:])
    sc2 = const.tile([1, 1], f32, tag="sc2", name="sc2")
    nc.sync.dma_start(sc2[:], sc2_d[:])

    ones_row = const.tile([1, P], bf16, tag="onesrow", name="onesrow")
    nc.vector.memset(ones_row[:], 1.0)
    ones_rf = const.tile([1, P], f32, tag="onesrf", name="onesrf")
    nc.vector.memset(ones_rf[:], 1.0)
    ones_col = const.tile([P, 1], bf16, tag="onescol", name="onescol")
    nc.vector.memset(ones_col[:], 1.0)
    ones8 = const.tile([P, 2, 16], fp8, tag="ones8", name="ones8")
    nc.vector.memset(ones8[:], 1.0)
    warm = const.tile([P, FH], bf16, tag="warm", name="warm")
    nc.vector.memset(warm[:], 0.0)
    scr1 = const.tile([1, 1], f32, tag="scr1", name="scr1")
    nc.vector.memset(scr1[:], 1.0)
    epsv = const.tile([1, 1], f32, tag="epsv", name="epsv")
    nc.vector.memset(epsv[:], LN_EPS)
    scr2 = svec.tile([1, 1], f32, tag="sv", name="scr2")
    # Act program order starts with Sqrt so the sqrt table set (which also
    # serves Square and Copy for all of phase A) loads at t~0.
    nc.scalar.activation(scr2[:], scr1[:], AF.Sqrt)

    # Second HWDGE queue (Activation engine) carries half the prologue DMA.
    caug = const.tile([P, NJ, 65], bf16, tag="caug", name="caug")
    nc.sync.dma_start(caug[:], caug_d[:].rearrange("p (j r) -> p j r",
                                                   j=NJ))
    w1x = const.tile([P, 2, D], fp8, tag="w1x", name="w1x")

    w2_bf = const.tile([P, NJ], bf16, tag="w2bf", name="w2bf")
    nc.vector.tensor_copy(w2_bf[:], pv[:, 16:32])

    # ---- phase A: kt DMA (4 transfers; W1 block 0 slots before the last
    # one so the gate is never W1-starved on the serialized DMA channel) ----
    kt4 = [persist.tile([P, 4, NSH], bf16, tag=f"kt4_{g}", name=f"kt4_{g}")
           for g in range(4)]
    kt_bf = [kt4[j // 4][:, j % 4, :] for j in range(NJ)]
    # z8[0..7]: fp8 K-pair tiles of z' = k*rstd; z8[8] ("x8") carries the
    # -mu*rstd correction row on partition 0 (paired with the w1x column
    # sums, it folds the LN mean subtraction into the fp8 matmul).
    z8 = [persist.tile([P, 2, NSH], fp8, tag=f"z8_{a}", name=f"z8_{a}")
          for a in range(NA)]
    x8 = persist.tile([P, 2, NSH], fp8, tag="x8", name="x8")
    gp.memset(x8[:], 0.0)
    G_sb = [persist.tile([64, FH], bf16, tag=f"gsb{h}", name=f"gsb{h}")
            for h in range(NH)]
    rstd_b = persist.tile([P, NSH], bf16, tag="rstdb", name="rstdb")
    mr_row = persist.tile([1, NSH], bf16, tag="mrrow", name="mrrow")

    w1pool = ctx.enter_context(tc.tile_pool(name="w1p", bufs=1))
    w14 = [w1pool.tile([P, 4, NJ, P], fp8, tag=f"w14_{g}", name=f"w14_{g}")
           for g in range(4)]

    mu = [svec.tile([1, FH], f32, tag="mu", bufs=2, name=f"mu{h}")
          for h in range(NH)]
    bt_bf = const.tile([64, D], bf16, tag="btbf", name="btbf")

    with tc.tile_pool(name="psA", bufs=1, space="PSUM") as psA:
        psum_G = [psA.tile([65, FH], f32, tag=f"psG{h}", name=f"psG{h}")
                  for h in range(NH)]
        psum_Q = [psA.tile([1, FH], f32, tag=f"psQ{h}", name=f"psQ{h}")
                  for h in range(NH)]

        # PE clock-ramp warmers while the first kt tiles are in flight
        # (psum_Q is re-start()ed by the real accumulation later).
        for _ in range(4):
            nc.tensor.matmul(psum_Q[0][:], ones_col[:], warm[:],
                             start=True, stop=True)

        def dma_kt(g):
            nc.sync.dma_start(kt4[g][:],
                              kt_d[g * 4 * P:(g + 1) * 4 * P, :]
                              .rearrange("(q p) n -> p q n", p=P))

        def dma_w14(g):
            nc.sync.dma_start(w14[g][:],
                              w18_d[g * 4 * P:(g + 1) * 4 * P, :]
                              .rearrange("(q p) d -> p q d", p=P))

        nc.sync.dma_start(w1x[:],
                          w1x_d[:].rearrange("p (t m) -> p t m", t=2))
        for g in range(3):
            dma_kt(g)
        dma_w14(0)
        dma_kt(3)
        for g in range(1, 4):
            dma_w14(g)
        nc.sync.dma_start(bt_bf[:], bt_d[:])
        for j in range(NJ):
            a, t = j // 2, j % 2
            st, sp = j == 0, j == NJ - 1
            # fp8 squares for the DoubleRow sumsq, spread over the three
            # otherwise-idle engines (Act via the Square activation)
            sq8 = (sq8p.tile([P, 2, NSH], fp8, tag="sq8", name="sq8")
                   if t == 0 else sq8)
            if j % 2 == 0:
                nc.scalar.square(sq8[:, t, :], kt_bf[j])
            elif j % 4 == 1:
                nc.vector.tensor_mul(sq8[:, t, :], kt_bf[j], kt_bf[j])
            else:
                gp.tensor_mul(sq8[:, t, :], kt_bf[j], kt_bf[j])
            for h in range(NH):
                sl = slice(h * FH, (h + 1) * FH)
                nc.tensor.matmul(psum_G[h][:], caug[:, j, :],
                                 kt_bf[j][:, sl], start=st, stop=sp)
            if t == 1:
                for h in range(NH):
                    sl = slice(h * FH, (h + 1) * FH)
                    nc.tensor.matmul(psum_Q[h][:], ones8[:, :, 0:1],
                                     sq8[:, :, sl], start=(a == 0),
                                     stop=(a == NA - 1), perf_mode=DR)



        # ---- LN stats finalize: mu -> mu^2 -> var -> sqrt(var+eps) -> 1/s --
        rstd_f = [svec.tile([1, FH], f32, tag="rstdf", bufs=2,
                            name=f"rstdf{h}") for h in range(NH)]
        mu2 = [svec.tile([1, FH], f32, tag="sv", name=f"mu2{h}")
               for h in range(NH)]
        var = [svec.tile([1, FH], f32, tag="var", bufs=2, name=f"var{h}")
               for h in range(NH)]
        sdev = [svec.tile([1, FH], f32, tag="sdev", bufs=2, name=f"sdev{h}")
                for h in range(NH)]
        for h in range(NH):
            nc.scalar.mul(mu[h][:], psum_G[h][64:65, :], 1.0 / D)
        for h in range(NH):
            nc.vector.tensor_mul(mu2[h][:], mu[h][:], mu[h][:])
            nc.vector.scalar_tensor_tensor(var[h][:], psum_Q[h][:], 1.0 / D,
                                           mu2[h][:], op0=ALU.mult,
                                           op1=ALU.subtract)
            nc.scalar.activation(sdev[h][:], var[h][:], AF.Sqrt,
                                 bias=epsv[0:1, 0:1])
            nc.vector.reciprocal(rstd_f[h][:], sdev[h][:])
        for h in range(NH):
            sl = slice(h * FH, (h + 1) * FH)
            nc.vector.tensor_mul(mr_row[0:1, sl], mu[h][:], rstd_f[h][:])
            # x-row for the matmul-folded mean subtraction: 8*mu*rstd in fp8
            # (pairs with w1x = -colsum/8)
            nc.scalar.mul(x8[0:1, 0, sl], mr_row[0:1, sl], 8.0)
            pb = psBC.tile([P, FH], f32, tag="pbc", name="pbc")
            nc.tensor.matmul(pb[:], ones_rf[0:1, 0:P], rstd_f[h][:],
                             start=True, stop=True)
            if h == 0:
                nc.scalar.copy(rstd_b[:, sl], pb[:])
            else:
                nc.vector.tensor_copy(rstd_b[:, sl], pb[:])
        for h in range(NH):
            nc.scalar.mul(G_sb[h][:], psum_G[h][0:64, :], SCALE)

    # silu table load happens here, hidden behind the first gate columns
    if not sim_safe:
        nc.scalar.activation(scr2[:], scr1[:], AF.Silu)

    # ---- z' = k * rstd in fp8, h-major, split across DVE and gpsimd -------
    for h in range(NH):
        sl = slice(h * FH, (h + 1) * FH)
        for j in range(NJ):
            eng = gp if j % 3 == 2 else nc.vector
            eng.tensor_mul(z8[j // 2][:, j % 2, sl],
                           kt_bf[j][:, sl], rstd_b[:, sl])
        del sl

    # ---- gate -------------------------------------------------------------
    GRP = 6  # open PSUM accumulation groups; K-streams so PE tracks z-prep

    def emit_silu(s1, o, h2):
        if sim_safe:
            sbt = h2pool.tile([P, FH], f32, tag="sb", name="sb")
            nc.scalar.activation(sbt[:], s1[:], AF.Identity,
                                 bias=pv[:, o:o + 1], scale=1.0 / W1_SCALE)
            sig = h2pool.tile([P, FH], f32, tag="sig", name="sig")
            nc.scalar.activation(sig[:], s1[:], AF.Sigmoid,
                                 bias=pv[:, o:o + 1], scale=1.0 / W1_SCALE)
            nc.vector.tensor_mul(h2[:], sbt[:], sig[:])
        else:
            nc.scalar.activation(h2[:], s1[:], AF.Silu,
                                 bias=pv[:, o:o + 1], scale=1.0 / W1_SCALE)

    with ExitStack() as gctx:
        psB = gctx.enter_context(tc.tile_pool(name="psB", bufs=GRP,
                                              space="PSUM"))
        psL = gctx.enter_context(tc.tile_pool(name="psL", bufs=1,
                                              space="PSUM"))
        psum_L = {}
        trow = {}
        Gw = {}

        def emit_gate_group(h, grp):
            sl = slice(h * FH, (h + 1) * FH)
            s1 = {}
            for o in grp:
                s1[o] = psB.tile([P, FH], f32, tag="s1", name="s1")
                # correction row first: its inputs are ready at stats time,
                # so it fills the PE gap before z' lands
                nc.tensor.matmul(s1[o][:], w1x[:, :, o * P:(o + 1) * P],
                                 x8[:, :, sl], start=True, stop=False,
                                 perf_mode=DR)
            for a in range(NA):
                for o in grp:
                    nc.tensor.matmul(
                        s1[o][:], w14[o // 4][:, o % 4, 2 * a:2 * a + 2, :],
                        z8[a][:, :, sl], start=False, stop=(a == NA - 1),
                        perf_mode=DR)
            for o in grp:
                h2 = h2pool.tile([P, FH], bf16, tag="h2", name="h2")
                emit_silu(s1[o], o, h2)
                nc.tensor.matmul(psum_L[h][:], w2_bf[:, o:o + 1], h2[:],
                                 start=(o == 0), stop=(o == NJ - 1))

        def emit_trow(h):
            # wv via tanh: sigmoid(x) = 0.5*tanh(x/2)+0.5 keeps the Act
            # engine inside the silu table set (no reload)
            trow[h] = svec.tile([1, FH], bf16, tag="trow", bufs=2,
                                name=f"trow{h}")
            nc.scalar.activation(trow[h][:], psum_L[h][:], AF.Tanh,
                                 bias=sc2[0:1, 0:1], scale=0.5)

        def emit_wv(h, pool, ptag):
            pw = pool.tile([P, FH], f32, tag=ptag, name="pw")
            nc.tensor.matmul(pw[0:64, :], ones_row[0:1, 0:64], trow[h][:],
                             start=True, stop=True)
            wcat = persist.tile([64, FH], bf16, tag=f"wcat{h}",
                                name=f"wcat{h}")
            nc.scalar.activation(wcat[0:32, :], pw[0:32, :], AF.Copy,
                                 bias=0.5, scale=0.5)
            nc.vector.tensor_scalar(wcat[32:64, :], pw[32:64, :], -0.5, 0.5,
                                    op0=ALU.mult, op1=ALU.add)
            Gw[h] = persist.tile([64, FH], bf16, tag=f"gw{h}", name=f"gw{h}")
            nc.vector.tensor_mul(Gw[h][:], G_sb[h][:], wcat[:])

        otp = {}

        def emit_tier(h, ms, pool, ptag, ob=2):
            """Tier-read matmuls for out row-blocks ms; output DMA batched
            ob row-blocks at a time."""
            for m in ms:
                pvt = pool.tile([P, FH], f32, tag=ptag, name="vt")
                nc.tensor.matmul(pvt[:], bt_bf[0:64, m * P:(m + 1) * P],
                                 Gw[h][:], start=True, stop=True)
                if m % ob == 0:
                    otp[h] = outpool.tile([P, ob, FH], bf16, tag=f"ot{ob}",
                                          name="ot")
                if m % 2 == 0:
                    nc.scalar.copy(otp[h][:, m % ob, :], pvt[:])
                else:
                    nc.vector.tensor_copy(otp[h][:, m % ob, :], pvt[:])
                if m % ob == ob - 1:
                    nc.sync.dma_start(
                        out_d[(m - ob + 1) * P:(m + 1) * P,
                              h * FH:(h + 1) * FH]
                        .rearrange("(q p) f -> p q f", p=P), otp[h][:])

        GB = list(range(0, NJ, GRP))
        for h in range(NH):
            psum_L[h] = psL.tile([1, FH], f32, tag="psL", name=f"psL{h}")

        # gate h0; tier h0 interleaves into gate h1's groups (its matmuls
        # slot between the z'-paced DoubleRow streams; copies ride Act/DVE)
        for g0 in GB:
            emit_gate_group(0, range(g0, min(g0 + GRP, NJ)))
        emit_trow(0)
        for gi, g0 in enumerate(GB):
            emit_gate_group(1, range(g0, min(g0 + GRP, NJ)))
            if gi == 0:
                emit_wv(0, psBC, "pbc")
                emit_tier(0, range(0, 6), psBC, "pbc")
            elif gi == 1:
                emit_tier(0, range(6, 12), psBC, "pbc")
            else:
                emit_tier(0, range(12, NJ), psBC, "pbc")
        emit_trow(1)
        emit_wv(1, psBC, "pbc")
        emit_tier(1, range(NJ), psB, "s1")


# ------------------------------------------------------------- legacy build

def build_nc_legacy(mode: str, has_d: bool, repeat: int = 1,
                    sim_safe: bool = False):
    import concourse.bacc as bacc
    import concourse.tile as tile
    from concourse import mybir

    f32 = mybir.dt.float32
    nc = bacc.Bacc("TRN2", target_bir_lowering=False, debug=False,
                   num_devices=NCORES)

    bf16 = mybir.dt.bfloat16
    kt_d = nc.dram_tensor("kt", [D, NSH], bf16, kind="ExternalInput")
    w1t_d = nc.dram_tensor("w1t", [D, D], bf16, kind="ExternalInput")
    pv_d = nc.dram_tensor("pv", [P, 64], f32, kind="ExternalInput")
    sc_d = nc.dram_tensor("sc", [1, 1], f32, kind="ExternalInput")
    bt_d = nc.dram_tensor("bt", [64, D], f32, kind="ExternalInput")
    caug_d = ct_d = dv_d = None
    if mode == "lowrank":
        caug_d = nc.dram_tensor("caug", [D, 65], f32, kind="ExternalInput")
    else:
        ct_d = nc.dram_tensor("ct", [64, D], f32, kind="ExternalInput")
    if has_d:
        dv_d = nc.dram_tensor("dv", [P, 32], f32, kind="ExternalInput")
    out_d = nc.dram_tensor("outT", [D, NSH], bf16, kind="ExternalOutput")

    with tile.TileContext(nc) as tc:
        for _ in range(repeat):
            with ExitStack() as ctx:
                _emit_legacy(ctx, tc, nc, mode, has_d,
                             kt_d, w1t_d, pv_d, sc_d, bt_d, caug_d, ct_d,
                             dv_d, out_d, sim_safe=sim_safe)
    nc.compile()
    return nc


def _emit_legacy(ctx, tc, nc, mode, has_d,
                 kt_d, w1t_d, pv_d, sc_d, bt_d, caug_d, ct_d, dv_d, out_d,
                 sim_safe=False):
    from concourse import mybir

    f32 = mybir.dt.float32
    bf16 = mybir.dt.bfloat16
    AF = mybir.ActivationFunctionType
    ALU = mybir.AluOpType
    lowrank = mode == "lowrank"

    const = ctx.enter_context(tc.tile_pool(name="const", bufs=1))
    persist = ctx.enter_context(tc.tile_pool(name="persist", bufs=1))
    tmp = ctx.enter_context(tc.tile_pool(name="tmp", bufs=3))
    kt2pool = ctx.enter_context(tc.tile_pool(name="kt2p", bufs=2))
    h2pool = ctx.enter_context(tc.tile_pool(name="h2p", bufs=2))
    outpool = ctx.enter_context(tc.tile_pool(name="outp", bufs=3))
    small = ctx.enter_context(tc.tile_pool(name="small", bufs=1))
    svec = ctx.enter_context(tc.tile_pool(name="svec", bufs=4))

    pv = const.tile([P, 64], f32, tag="pv", name="pv")
    nc.sync.dma_start(pv[:], pv_d[:])
    sc = const.tile([1, 1], f32, tag="sc", name="sc")
    nc.sync.dma_start(sc[:], sc_d[:])

    if lowrank:
        caug_v = caug_d[:].rearrange("(j p) r -> p j r", p=P)
        with tc.tile_pool(name="caugstage", bufs=1) as caugstage:
            cf3 = caugstage.tile([P, NJ, 65], f32, tag="caugf", name="caugf")
            nc.sync.dma_start(cf3[:], caug_v[:])
            cb3 = const.tile([P, NJ, 65], bf16, tag="caugbf", name="caugbf")
            nc.vector.tensor_copy(cb3[:], cf3[:])
        caug_bf = [cb3[:, j, :] for j in range(NJ)]
    ones_col = const.tile([P, 1], bf16, tag="ones", name="ones")
    nc.vector.memset(ones_col[:], 1.0)
    ones_row = const.tile([1, P], bf16, tag="onesrow", name="onesrow")
    nc.vector.memset(ones_row[:], 1.0)
    ones_rf = const.tile([1, P], f32, tag="onesrf", name="onesrf")
    nc.vector.memset(ones_rf[:], 1.0)
    psBC = ctx.enter_context(tc.tile_pool(name="psBC", bufs=1, space="PSUM"))

    def bcast_psum(src_row_bf16, nparts):
        pb = psBC.tile([nparts, FH], f32, tag="pbc", name="pbc")
        nc.tensor.matmul(pb[:], ones_row[0:1, 0:nparts], src_row_bf16[:],
                         start=True, stop=True)
        return pb

    w2_bf = const.tile([P, NJ], bf16, tag="w2bf", name="w2bf")
    nc.vector.tensor_copy(w2_bf[:], pv[:, 48:64])

    if has_d:
        dv = const.tile([P, 32], f32, tag="dv", name="dv")
        nc.sync.dma_start(dv[:], dv_d[:])

    htpool = ctx.enter_context(tc.tile_pool(name="htp", bufs=1))
    ktpool = persist
    kt_bf = [ktpool.tile([P, NSH], bf16, tag=f"ktbf{j}", name=f"ktbf{j}") for j in range(NJ)]
    mu = [small.tile([1, FH], f32, tag=f"mu{h}", name=f"mu{h}") for h in range(NH)]
    msq = [small.tile([1, FH], f32, tag=f"msq{h}", name=f"msq{h}") for h in range(NH)]
    G_sb = None
    if lowrank:
        G_sb = [persist.tile([64, FH], bf16, tag=f"gsb{h}", name=f"gsb{h}") for h in range(NH)]

    with tc.tile_pool(name="psA", bufs=1, space="PSUM") as psA:
        if lowrank:
            psum_G = [psA.tile([65, FH], f32, tag=f"psG{h}", name=f"psG{h}") for h in range(NH)]
        else:
            psum_S = [psA.tile([1, FH], f32, tag=f"psS{h}", name=f"psS{h}") for h in range(NH)]
        psum_Q = [psA.tile([1, FH], f32, tag=f"psQ{h}", name=f"psQ{h}") for h in range(NH)]

        for j in range(NJ):
            st, sp = j == 0, j == NJ - 1
            nc.sync.dma_start(kt_bf[j][:], kt_d[j * P:(j + 1) * P, :])
            kt2 = kt2pool.tile([P, NSH], bf16, tag="kt2", name="kt2")
            nc.vector.tensor_mul(kt2[:], kt_bf[j][:], kt_bf[j][:])
            for h in range(NH):
                sl = slice(h * FH, (h + 1) * FH)
                if lowrank:
                    nc.tensor.matmul(psum_G[h][:], caug_bf[j][:],
                                     kt_bf[j][:, sl], start=st, stop=sp)
                else:
                    nc.tensor.matmul(psum_S[h][:], ones_col[:],
                                     kt_bf[j][:, sl], start=st, stop=sp)
                nc.tensor.matmul(psum_Q[h][:], ones_col[:],
                                 kt2[:, sl], start=st, stop=sp)

        for h in range(NH):
            if lowrank:
                nc.scalar.mul(G_sb[h][:], psum_G[h][0:64, :], SCALE)
                nc.scalar.mul(mu[h][:], psum_G[h][64:65, :], 1.0 / D)
            else:
                nc.scalar.mul(mu[h][:], psum_S[h][:], 1.0 / D)
            nc.scalar.mul(msq[h][:], psum_Q[h][:], 1.0 / D)

    if lowrank:
        bt_bf = const.tile([64, D], bf16, tag="btbf", name="btbf")
    else:
        btt_bf = [const.tile([32, D], bf16, tag=f"btbf{t}", name=f"btbf{t}")
                  for t in range(2)]
        ctt_bf = [const.tile([32, D], bf16, tag=f"ctbf{t}", name=f"ctbf{t}")
                  for t in range(2)]
    with tc.tile_pool(name="facstage", bufs=2) as facstage:
        for q in range(2):
            qs = slice(q * (D // 2), (q + 1) * (D // 2))
            if lowrank:
                btf = facstage.tile([64, D // 2], f32, tag="btf", name="btf")
                nc.sync.dma_start(btf[:], bt_d[:, qs])
                nc.vector.tensor_copy(bt_bf[:, qs], btf[:])
            else:
                for t in range(2):
                    btf = facstage.tile([32, D // 2], f32, tag="btf", name="btf")
                    nc.sync.dma_start(btf[:], bt_d[32 * t:32 * t + 32, qs])
                    nc.vector.tensor_copy(btt_bf[t][:, qs], btf[:])
                    ctf = facstage.tile([32, D // 2], f32, tag="ctf", name="ctf")
                    nc.sync.dma_start(ctf[:], ct_d[32 * t:32 * t + 32, qs])
                    nc.vector.tensor_copy(ctt_bf[t][:, qs], ctf[:])

    mu_b = [persist.tile([P, FH], bf16, tag=f"mub{h}", name=f"mub{h}") for h in range(NH)]
    rstd_b = [persist.tile([P, FH], bf16, tag=f"rstdb{h}", name=f"rstdb{h}") for h in range(NH)]
    for h in range(NH):
        mu2 = svec.tile([1, FH], f32, tag="sv", name="mu2")
        nc.vector.tensor_mul(mu2[:], mu[h][:], mu[h][:])
        veps = svec.tile([1, FH], f32, tag="sv", name="veps")
        nc.vector.scalar_tensor_tensor(veps[:], msq[h][:], LN_EPS, mu2[:],
                                       op0=ALU.add, op1=ALU.subtract)
        rinv = svec.tile([1, FH], f32, tag="sv", name="rinv")
        nc.vector.reciprocal(rinv[:], veps[:])
        rstd_bf = svec.tile([1, FH], bf16, tag="sv", name="rstd_bf")
        nc.scalar.activation(rstd_bf[:], rinv[:], AF.Sqrt)
        mu_bf = svec.tile([1, FH], bf16, tag="sv", name="mu_bf")
        nc.scalar.copy(mu_bf[:], mu[h][:])
        nc.scalar.copy(mu_b[h][:], bcast_psum(mu_bf, P)[:])
        nc.scalar.copy(rstd_b[h][:], bcast_psum(rstd_bf, P)[:])

    wv = [svec.tile([1, FH], f32, tag="wvlong", bufs=2, name=f"wv{h}")
          for h in range(NH)]
    w1t_v = w1t_d[:].rearrange("(j p) o -> p j o", p=P)

    def emit_ln():
        ht = [htpool.tile([P, NSH], bf16, tag=f"ht{j}", name=f"ht{j}")
              for j in range(NJ)]
        for h in range(NH):
            for j in range(NJ):
                sl = slice(h * FH, (h + 1) * FH)
                t1 = tmp.tile([P, FH], bf16, tag="lnt1", name="lnt1")
                nc.vector.tensor_sub(t1[:], kt_bf[j][:, sl], mu_b[h][:])
                t2 = tmp.tile([P, FH], bf16, tag="lnt2", name="lnt2")
                nc.vector.tensor_mul(t2[:], t1[:], rstd_b[h][:])
                nc.scalar.activation(ht[j][:, sl], t2[:], AF.Identity,
                                     bias=pv[:, 16 + j:17 + j],
                                     scale=pv[:, j:j + 1])
        return ht

    def emit_silu(s1, o, h2):
        if sim_safe:
            sbt = h2pool.tile([P, FH], f32, tag="sb", name="sb")
            nc.scalar.activation(sbt[:], s1[:], AF.Identity,
                                 bias=pv[:, 32 + o:33 + o])
            sig = h2pool.tile([P, FH], f32, tag="sig", name="sig")
            nc.scalar.activation(sig[:], s1[:], AF.Sigmoid,
                                 bias=pv[:, 32 + o:33 + o])
            nc.vector.tensor_mul(h2[:], sbt[:], sig[:])
        else:
            nc.scalar.activation(h2[:], s1[:], AF.Silu,
                                 bias=pv[:, 32 + o:33 + o])

    def emit_gate_col(psB, psum_L, w1b_j_aps, o, h):
        sl = slice(h * FH, (h + 1) * FH)
        s1 = psB.tile([P, FH], f32, tag="s1", name="s1")
        for j in range(NJ):
            nc.tensor.matmul(s1[:], w1b_j_aps[j], ht[j][:, sl],
                             start=(j == 0), stop=(j == NJ - 1))
        h2 = h2pool.tile([P, FH], bf16, tag="h2", name="h2")
        emit_silu(s1, o, h2)
        nc.tensor.matmul(psum_L[h][:], w2_bf[:, o:o + 1], h2[:],
                         start=(o == 0), stop=(o == NJ - 1))

    def emit_tier_lowrank(h, psC):
        wv_bf = svec.tile([1, FH], bf16, tag="sv", name="wv_bf")
        nc.vector.tensor_copy(wv_bf[:], wv[h][:])
        nb = P if has_d else 64
        pw = bcast_psum(wv_bf, nb)
        wcat = persist.tile([64, FH], bf16, tag=f"wcat{h}", name=f"wcat{h}")
        nc.scalar.copy(wcat[0:32, :], pw[0:32, :])
        nc.scalar.activation(wcat[32:64, :], pw[32:64, :], AF.Copy,
                             bias=1.0, scale=-1.0)
        if has_d:
            wb = persist.tile([P, FH], bf16, tag=f"wb128{h}", name=f"wb128{h}")
            nc.scalar.copy(wb[:], pw[:])
        Gw = persist.tile([64, FH], bf16, tag=f"gw{h}", name=f"gw{h}")
        nc.vector.tensor_mul(Gw[:], G_sb[h][:], wcat[:])
        for m in range(NJ):
            pvt = psC.tile([P, FH], f32, tag="vt", name="vt")
            nc.tensor.matmul(pvt[:], bt_bf[0:64, m * P:(m + 1) * P],
                             Gw[:], start=True, stop=True)
            ot = outpool.tile([P, FH], bf16, tag="ot", name="ot")
            if not has_d:
                if m % 2 == 0:
                    nc.scalar.copy(ot[:], pvt[:])
                else:
                    nc.vector.tensor_copy(ot[:], pvt[:])
            else:
                sl = slice(h * FH, (h + 1) * FH)
                dmix = tmp.tile([P, FH], bf16, tag="dmix", name="dmix")
                nc.vector.tensor_scalar(dmix[:], wb[:],
                                        dv[:, m:m + 1], dv[:, 16 + m:17 + m],
                                        op0=ALU.mult, op1=ALU.add)
                c = tmp.tile([P, FH], f32, tag="dc", name="dc")
                nc.vector.tensor_mul(c[:], kt_bf[m][:, sl], dmix[:])
                nc.vector.tensor_add(ot[:], pvt[:], c[:])
            nc.sync.dma_start(
                out_d[m * P:(m + 1) * P, h * FH:(h + 1) * FH], ot[:])

    ht = emit_ln()
    with ExitStack() as gctx:
        w1bp = gctx.enter_context(tc.tile_pool(name="w1bp", bufs=2))
        psC = gctx.enter_context(tc.tile_pool(name="psC", bufs=2,
                                              space="PSUM"))
        with tc.tile_pool(name="psB", bufs=2, space="PSUM") as psB, \
             tc.tile_pool(name="psL", bufs=1, space="PSUM") as psL:
            psum_L = [psL.tile([1, FH], f32, tag=f"psL{h}",
                               name=f"psL{h}") for h in range(NH)]
            for o in range(NJ):
                w1b = w1bp.tile([P, NJ, P], bf16, tag="w1b", name="w1b")
                nc.sync.dma_start(w1b[:], w1t_v[:, :, o * P:(o + 1) * P])
                for h in range(NH):
                    aps = [w1b[:, j, :] for j in range(NJ)]
                    emit_gate_col(psB, psum_L, aps, o, h)
            for h in range(NH):
                nc.scalar.activation(wv[h][:], psum_L[h][:], AF.Sigmoid,
                                     bias=sc[0:1, 0:1])
            if lowrank:
                for h in range(NH):
                    emit_tier_lowrank(h, psC)

    if not lowrank:
        wpb = [persist.tile([P, FH], f32, tag=f"wpb{h}", name=f"wpb{h}") for h in range(NH)]
        wqb = [persist.tile([P, FH], f32, tag=f"wqb{h}", name=f"wqb{h}") for h in range(NH)]
        wb128 = []
        for h in range(NH):
            wv_bf = svec.tile([1, FH], bf16, tag="sv", name="wv_bf")
            nc.vector.tensor_copy(wv_bf[:], wv[h][:])
            pw = bcast_psum(wv_bf, P)
            nc.scalar.mul(wpb[h][:], pw[:], SCALE)
            nc.scalar.activation(wqb[h][:], pw[:], AF.Copy,
                                 bias=SCALE, scale=-SCALE)
            if has_d:
                wb = persist.tile([P, FH], bf16, tag=f"wb128{h}", name=f"wb128{h}")
                nc.scalar.copy(wb[:], pw[:])
                wb128.append(wb)

        with ExitStack() as tctx:
            mpool = tctx.enter_context(tc.tile_pool(name="mtiles", bufs=1))
            psD = tctx.enter_context(tc.tile_pool(name="psD", bufs=2,
                                                  space="PSUM"))
            for mg in range(D // FH):
                mt = [[], []]
                for t in range(2):
                    for j in range(NJ):
                        pm = psD.tile([P, FH], f32, tag="pm", name="pm",
                                      bufs=1)
                        nc.tensor.matmul(
                            pm[:], ctt_bf[t][:, j * P:(j + 1) * P],
                            btt_bf[t][:, mg * FH:(mg + 1) * FH],
                            start=True, stop=True)
                        mtile = mpool.tile([P, FH], bf16, tag=f"m{t}_{j}", name=f"m{t}_{j}")
                        nc.scalar.activation(mtile[:], pm[:], AF.Tanh)
                        mt[t].append(mtile)
                for s in range(FH // P):
                    m = mg * (FH // P) + s
                    for h in range(NH):
                        sl = slice(h * FH, (h + 1) * FH)
                        pf = psD.tile([P, FH], f32, tag="pf", name="pf")
                        for j in range(NJ):
                            nc.tensor.matmul(pf[:],
                                             mt[0][j][:, s * P:(s + 1) * P],
                                             kt_bf[j][:, sl],
                                             start=(j == 0), stop=(j == NJ - 1))
                        pd_ = psD.tile([P, FH], f32, tag="pd", name="pd")
                        for j in range(NJ):
                            nc.tensor.matmul(pd_[:],
                                             mt[1][j][:, s * P:(s + 1) * P],
                                             kt_bf[j][:, sl],
                                             start=(j == 0), stop=(j == NJ - 1))
                        t0 = tmp.tile([P, FH], f32, tag="t0", name="t0")
                        nc.vector.tensor_mul(t0[:], pf[:], wpb[h][:])
                        t1 = tmp.tile([P, FH], f32, tag="t1", name="t1")
                        nc.vector.tensor_mul(t1[:], pd_[:], wqb[h][:])
                        ot = outpool.tile([P, FH], bf16, tag="ot", name="ot")
                        nc.vector.tensor_add(ot[:], t0[:], t1[:])
                        if has_d:
                            dmix = tmp.tile([P, FH], bf16, tag="dmix", name="dmix")
                            nc.vector.tensor_scalar(dmix[:], wb128[h][:],
                                                    dv[:, m:m + 1],
                                                    dv[:, 16 + m:17 + m],
                                                    op0=ALU.mult, op1=ALU.add)
                            c = tmp.tile([P, FH], f32, tag="dc", name="dc")
                            nc.vector.tensor_mul(c[:], kt_bf[m][:, sl], dmix[:])
                            ot2 = outpool.tile([P, FH], bf16, tag="ot2",
                                               name="ot2")
                            nc.vector.tensor_add(ot2[:], ot[:], c[:])
                            ot = ot2
                        nc.sync.dma_start(
                            out_d[m * P:(m + 1) * P, h * FH:(h + 1) * FH],
                            ot[:])


# ---------------------------------------------------------------- host side

def _chunked(vec):
    """[2048] -> [128, 16]; column j holds elements j*128 .. j*128+127."""
    return np.ascontiguousarray(np.asarray(vec, np.float32).reshape(NJ, P).T)


def _pick_mode(fast_B, fast_C, deep_B, deep_C):
    worst = 0.0
    for B, C in ((fast_B, fast_C), (deep_B, deep_C)):
        bound = (np.linalg.norm(B, axis=1).max() *
                 np.linalg.norm(C, axis=1).max())
        if bound > LOWRANK_THR:
            bound = float(np.abs(B @ C.T).max())
        worst = max(worst, float(bound))
    return "lowrank" if worst <= LOWRANK_THR else "tanh"


def prepare(inputs):
    """-> (mode, has_d, in_maps).  mode 'fast' selects the fp8 kernel."""
    import ml_dtypes
    bf = ml_dtypes.bfloat16
    f8 = ml_dtypes.float8_e4m3

    g = {k: np.asarray(v, np.float32) for k, v in inputs.items()}
    k = g["k"]
    assert k.shape == (N, D), k.shape

    mode = _pick_mode(g["fast_B"], g["fast_C"], g["deep_B"], g["deep_C"])
    has_d = bool(np.any(g["fast_D"]) or np.any(g["deep_D"]))
    fast = mode == "lowrank" and not has_d

    if fast:
        # LN affine folded into the gate weights: W1' = W1*diag(gamma),
        # b1' = b1 + W1 @ beta.  W1 pre-scaled by 32 for fp8.
        W1p = g["gate_W1"] * g["ln_gamma"][None, :]
        b1p = g["gate_b1"] + g["gate_W1"] @ g["ln_beta"]
        W1s8 = (W1p * W1_SCALE).astype(f8)
        # lhsT o-block tiles [p, j, m] = W1s[o*128+m, j*128+p], stored so each
        # partition row is 2048 contiguous bytes (full-BW DMA).
        A = np.ascontiguousarray(
            W1s8.reshape(NJ, P, NJ, P).transpose(0, 3, 2, 1))  # [o, p, j, m]
        w18 = A.reshape(NJ * P, D)
        # mean-subtraction fold: stationary x-row tile [P, 2, D] with only
        # partition 0 / plane 0 live = -colsum(W1s8)/8 (pairs with the
        # device-written 8*mu*rstd moving row; product = -c * mu * rstd)
        c8 = (-W1s8.astype(np.float32).sum(axis=1) / 8.0).astype(f8)
        w1x = np.zeros((P, 2 * D), f8)
        w1x[0, 0:D] = c8
        caug_np = np.concatenate(
            [g["fast_C"], g["deep_C"], np.ones((D, 1), np.float32)],
            axis=1)  # [D, 65]
        caugp = np.ascontiguousarray(
            caug_np.reshape(NJ, P, 65).transpose(1, 0, 2).reshape(P, NJ * 65))
        pv = np.concatenate([_chunked(b1p), _chunked(g["gate_W2"][0])],
                            axis=1)
        common = {
            "w18": w18,
            "w1x": w1x,
            "caug": caugp.astype(bf),
            "pv": pv,
            "sc2": np.array([[0.5 * (g["gate_b2"][0] + g["base_logit"][0])]],
                            np.float32),
            "bt": np.ascontiguousarray(
                np.concatenate([g["fast_B"].T, g["deep_B"].T],
                               axis=0)).astype(bf),
        }
        in_maps = []
        for i in range(NCORES):
            m = dict(common)
            m["kt"] = np.ascontiguousarray(
                k[i * NSH:(i + 1) * NSH, :].T).astype(bf)
            in_maps.append(m)
        return "fast", has_d, in_maps

    pv = np.concatenate([_chunked(g["ln_gamma"]), _chunked(g["ln_beta"]),
                         _chunked(g["gate_b1"]), _chunked(g["gate_W2"][0])],
                        axis=1)
    common = {
        "w1t": np.ascontiguousarray(g["gate_W1"].T).astype(bf),
        "pv": pv,
        "sc": np.array([[g["gate_b2"][0] + g["base_logit"][0]]], np.float32),
        "bt": np.ascontiguousarray(
            np.concatenate([g["fast_B"].T, g["deep_B"].T], axis=0)),
    }
    if mode == "lowrank":
        common["caug"] = np.ascontiguousarray(
            np.concatenate([g["fast_C"], g["deep_C"],
                            np.ones((D, 1), np.float32)], axis=1))
    else:
        common["ct"] = np.ascontiguousarray(
            np.concatenate([g["fast_C"].T, g["deep_C"].T], axis=0))
    if has_d:
        common["dv"] = np.ascontiguousarray(
            np.concatenate([_chunked(g["fast_D"] - g["deep_D"]),
                            _chunked(g["deep_D"])], axis=1))

    in_maps = []
    for i in range(NCORES):
        m = dict(common)
        m["kt"] = np.ascontiguousarray(
            k[i * NSH:(i + 1) * NSH, :].T).astype(ml_dtypes.bfloat16)
        in_maps.append(m)
    return mode, has_d, in_maps


def get_nc(mode, has_d, repeat=1, sim_safe=False):
    key = (mode, has_d, repeat, sim_safe)
    if key not in _NC_CACHE:
        if mode == "fast":
            _NC_CACHE[key] = build_nc_fast(repeat, sim_safe)
        else:
            _NC_CACHE[key] = build_nc_legacy(mode, has_d, repeat, sim_safe)
    return _NC_CACHE[key]


def assemble(results):
    out = np.empty((N, D), np.float32)
    for i in range(NCORES):
        out[i * NSH:(i + 1) * NSH, :] = results[i]["outT"].astype(np.float32).T
    return out


def kernel(**inputs) -> np.ndarray:
    from concourse.bass_utils import run_bass_kernel_spmd

    mode, has_d, in_maps = prepare(inputs)
    nc = get_nc(mode, has_d)
    res = run_bass_kernel_spmd(nc, in_maps, core_ids=list(range(NCORES)))
    return assemble(res.results)
